# revision 2
# baseline (speedup 1.0000x reference)
"""Trainium2 Bass kernel for the ChessTransformer problem, v2.

v2 restructure vs baseline: one For_i loop over sample tiles with embed +
all 12 transformer layers + head-token extraction inside the body, so the
activation tile never round-trips through HBM between layers (x stays in
SBUF). Layer weights are streamed per (tile, layer), double-buffered.
Matmul moving operands read the f32 x tile directly as float32r (1
cycle/row at >=256 free cols), removing all bf16 shadow copies of x.
"""

import sys

sys.path.insert(0, "/opt/trn_rl_repo")

import numpy as np
import ml_dtypes

import concourse.bacc as bacc
import concourse.bass as bass
import concourse.mybir as mybir
from concourse import tile
from concourse.bass_utils import run_bass_kernel_spmd

F32 = mybir.dt.float32
R32 = mybir.dt.float32r
BF16 = mybir.dt.bfloat16
AF = mybir.ActivationFunctionType
ALU = mybir.AluOpType

D = 1024
H = 8
DH = 128
T = 71
KV = 81  # 17 fen rows + 64 pos rows
G = 7  # samples per compute tile (G*T = 497 <= 512 PSUM cols)
N_CORES = 8
EPS = 1e-5


def _bf(a):
    return np.ascontiguousarray(a.astype(ml_dtypes.bfloat16))


def _f32(a):
    return np.ascontiguousarray(a.astype(np.float32))


def host_prep(inputs, n_cores=N_CORES):
    """Build per-core input maps + flags from full-size inputs."""
    fen = np.asarray(inputs["fen"]).astype(np.int64)
    move = np.asarray(inputs["move"]).astype(np.int64)
    B = fen.shape[0]
    Bc = B // n_cores
    L = np.asarray(inputs["qkv"]).shape[0]

    rank_emb = np.asarray(inputs["rank_emb"], np.float32)
    file_emb = np.asarray(inputs["file_emb"], np.float32)
    fen_emb = np.asarray(inputs["fen_emb"], np.float32)
    move_emb = np.asarray(inputs["move_emb"], np.float32)
    abs_emb = np.asarray(inputs["abs_emb"], np.float32)
    qkv = np.asarray(inputs["qkv"], np.float32)
    ff1 = np.asarray(inputs["ff1"], np.float32)
    ff2 = np.asarray(inputs["ff2"], np.float32)
    W1 = np.asarray(inputs["W1"], np.float32)
    b1 = np.asarray(inputs["b1"], np.float32)
    W2 = np.asarray(inputs["W2"], np.float32)
    b2 = np.asarray(inputs["b2"], np.float32)
    lng = np.asarray(inputs["ln_emb_g"], np.float32)
    lnb = np.asarray(inputs["ln_emb_b"], np.float32)
    log = np.asarray(inputs["ln_out_g"], np.float32)
    lob = np.asarray(inputs["ln_out_b"], np.float32)

    pos = (rank_emb + file_emb).reshape(64, D)

    vtab = np.concatenate([fen_emb, 0.58 * pos], axis=0)  # (81, D)
    C = np.empty((T, D), np.float32)
    C[:64] = 0.5 * pos + abs_emb[:64]
    C[64:69] = abs_emb[64:69]
    C[69:71] = 0.58 * move_emb + abs_emb[69:71]

    cnt = np.zeros((KV, B, T), np.float32)
    bidx = np.arange(B)[:, None]
    np.add.at(cnt, (fen[:, :64], bidx, np.arange(64)[None, :]), 0.5)
    np.add.at(cnt, (fen[:, 64:128], bidx, np.arange(64)[None, :]), 0.5)
    np.add.at(cnt, (fen[:, 128:133], bidx, np.arange(64, 69)[None, :]), 1.0)
    np.add.at(cnt, (17 + move, bidx, np.arange(69, 71)[None, :]), 1.0)
    cnt = cnt.reshape(KV, B * T)

    Cfm = C.T.reshape(8, 128, T)  # feature-major d-tiles
    cstr = np.tile(Cfm, (1, 1, G))

    scale = np.sqrt(np.float32(DH))
    wq = (qkv[:, 0] / scale).transpose(0, 2, 1, 3).reshape(L, 128, H * 128)
    wk = qkv[:, 1].transpose(0, 2, 1, 3).reshape(L, 128, H * 128)
    wv = qkv[:, 2].transpose(0, 2, 1, 3).reshape(L, 128, H * 128)
    wf1 = (
        ff1.reshape(L, H, 8, 128, DH).transpose(0, 3, 1, 2, 4).reshape(L, 128, H * 8 * 128)
    )
    wf2 = ff2.transpose(0, 2, 1, 3).reshape(L, 128, H * 128)

    w1t = W1.T.reshape(16, 128, 2 * D)
    w2s = W2.reshape(16, 128).T
    hb1 = b1.reshape(16, 128).T
    hg = log.reshape(16, 128).T
    hbt = lob.reshape(16, 128).T
    gemb = lng.reshape(8, 128).T
    bemb = lnb.reshape(8, 128).T

    flags = dict(
        apply_gemb=not (np.all(lng == 1.0) and np.all(lnb == 0.0)),
        apply_ghead=not (np.all(log == 1.0) and np.all(lob == 0.0)),
        use_b1=bool(np.any(b1 != 0.0)),
        use_b2=bool(np.any(b2 != 0.0)),
        use_prelu=True,
        Bc=Bc,
        L=L,
    )

    shared = {
        "vtab": _bf(vtab),
        "cstr": _f32(cstr),
        "wq": _bf(wq),
        "wk": _bf(wk),
        "wv": _bf(wv),
        "wf1": _bf(wf1),
        "wf2": _bf(wf2),
        "w1t": _bf(w1t),
        "w2s": _bf(w2s),
        "hb1": _f32(hb1),
        "hb1s": _f32(0.2 * hb1),
        "hg": _f32(hg),
        "hbt": _f32(hbt),
        "gemb": _f32(gemb),
        "bemb": _f32(bemb),
        "b2": _f32(b2.reshape(1, 1)),
    }
    cnt_bf = _bf(cnt)
    in_maps = []
    for c in range(n_cores):
        m = dict(shared)
        m["cnt"] = np.ascontiguousarray(cnt_bf[:, c * Bc * T : (c + 1) * Bc * T])
        in_maps.append(m)
    return in_maps, flags


def build_program(flags):
    Bc = flags["Bc"]
    L = flags["L"]
    TOK = Bc * T
    NT = Bc // G
    REM = Bc - NT * G
    NF = G * T  # 497
    NR = REM * T

    nc = bacc.Bacc("TRN2", target_bir_lowering=False, debug=False)

    cnt_d = nc.dram_tensor("cnt", [KV, TOK], BF16, kind="ExternalInput")
    vtab_d = nc.dram_tensor("vtab", [KV, D], BF16, kind="ExternalInput")
    cstr_d = nc.dram_tensor("cstr", [8, 128, NF], F32, kind="ExternalInput")
    wq_d = nc.dram_tensor("wq", [L, 128, H * 128], BF16, kind="ExternalInput")
    wk_d = nc.dram_tensor("wk", [L, 128, H * 128], BF16, kind="ExternalInput")
    wv_d = nc.dram_tensor("wv", [L, 128, H * 128], BF16, kind="ExternalInput")
    wf1_d = nc.dram_tensor("wf1", [L, 128, H * 8 * 128], BF16, kind="ExternalInput")
    wf2_d = nc.dram_tensor("wf2", [L, 128, H * 128], BF16, kind="ExternalInput")
    w1t_d = nc.dram_tensor("w1t", [16, 128, 2 * D], BF16, kind="ExternalInput")
    w2s_d = nc.dram_tensor("w2s", [128, 16], BF16, kind="ExternalInput")
    hb1_d = nc.dram_tensor("hb1", [128, 16], F32, kind="ExternalInput")
    hb1s_d = nc.dram_tensor("hb1s", [128, 16], F32, kind="ExternalInput")
    hg_d = nc.dram_tensor("hg", [128, 16], F32, kind="ExternalInput")
    hbt_d = nc.dram_tensor("hbt", [128, 16], F32, kind="ExternalInput")
    gemb_d = nc.dram_tensor("gemb", [128, 8], F32, kind="ExternalInput")
    bemb_d = nc.dram_tensor("bemb", [128, 8], F32, kind="ExternalInput")
    b2_d = nc.dram_tensor("b2", [1, 1], F32, kind="ExternalInput")
    out_d = nc.dram_tensor("out", [1, Bc], F32, kind="ExternalOutput")

    u_d = nc.dram_tensor("u", [16, 128, Bc], F32, kind="Internal")

    def r32(ap):
        return ap.bitcast(R32)

    with tile.TileContext(nc) as tc:
        with tc.tile_pool(name="const", bufs=1) as cpool:
            ones71 = cpool.tile([71, 128], BF16)
            nc.vector.memset(ones71[:], 1.0)
            ones128 = cpool.tile([128, 128], BF16)
            nc.vector.memset(ones128[:], 1.0)
            epsT = cpool.tile([128, 1], F32)
            nc.vector.memset(epsT[:], EPS)
            al02 = cpool.tile([128, 1], F32)
            nc.vector.memset(al02[:], 0.2)

            def leaky(out_ap, in_ap, bias=0.0):
                if flags.get("use_prelu", True):
                    nc.scalar.activation(
                        out_ap, in_ap, AF.Prelu, bias=bias, alpha=al02[: in_ap.shape[0]]
                    )
                else:
                    t_ = cpool.tile([128, out_ap.shape[1]], F32, tag="lk")
                    p_ = t_[: in_ap.shape[0], :]
                    nc.scalar.activation(p_, in_ap, AF.Copy, scale=0.2)
                    if isinstance(bias, float):
                        nc.any.tensor_tensor(out_ap, p_, in_ap, ALU.max)
                    else:
                        s_ = cpool.tile([128, out_ap.shape[1]], F32, tag="lk2")
                        s2 = s_[: in_ap.shape[0], :]
                        nc.vector.tensor_scalar_add(s2, in_ap, bias)
                        nc.scalar.activation(p_, s2, AF.Copy, scale=0.2)
                        nc.any.tensor_tensor(out_ap, p_, s2, ALU.max)

            # ---------------- fused embed + layers + extract ----------------
            with (
                tc.tile_pool(name="res", bufs=1) as res,
                tc.tile_pool(name="xres", bufs=1) as xres,
                tc.tile_pool(name="lw", bufs=2) as lw,
                tc.tile_pool(name="lsb", bufs=2) as lsb,
                tc.tile_pool(name="big", bufs=2) as big,
                tc.tile_pool(name="ps3", bufs=3, space="PSUM") as ps3,
                tc.tile_pool(name="ps2", bufs=3, space="PSUM") as ps2,
                tc.tile_pool(name="psv", bufs=1, space="PSUM") as psv,
            ):
                vtab_sb = res.tile([KV, D], BF16)
                nc.sync.dma_start(vtab_sb[:], vtab_d[:])
                cstr_sb = res.tile([128, 8 * NF], F32)
                for k in range(8):
                    nc.sync.dma_start(cstr_sb[:, k * NF : (k + 1) * NF], cstr_d[k])
                if flags["apply_gemb"]:
                    gemb_sb = res.tile([128, 8], F32)
                    nc.sync.dma_start(gemb_sb[:], gemb_d[:])
                    bemb_sb = res.tile([128, 8], F32)
                    nc.sync.dma_start(bemb_sb[:], bemb_d[:])

                def tile_body(cols, ucols, N, Gn):
                    x = xres.tile([128, 8 * NF], F32, tag="x")
                    xa = xres.tile([128, 8 * NF], F32, tag="xa")
                    xab = xres.tile([128, 8 * NF], BF16, tag="xab")

                    # ---- embed into x ----
                    cnt_t = lsb.tile([KV, NF], BF16, tag="cnt")
                    nc.sync.dma_start(cnt_t[:, :N], cnt_d[:, cols])
                    xp = xres.tile([128, 8 * NF], F32, tag="xp")
                    for k in range(8):
                        e_ps = ps2.tile([128, NF], F32, tag="acc")
                        nc.tensor.matmul(
                            e_ps[:, :N], vtab_sb[:, k * 128 : (k + 1) * 128],
                            cnt_t[:, :N], start=True, stop=True,
                        )
                        nc.any.tensor_tensor(
                            xp[:, k * NF : k * NF + N], e_ps[:, :N],
                            cstr_sb[:, k * NF : k * NF + N], ALU.add,
                        )
                    xbt = xres.tile([128, 8 * NF], BF16, tag="xbt")
                    for k in range(8):
                        nc.any.tensor_copy(
                            xbt[:, k * NF : k * NF + N], xp[:, k * NF : k * NF + N]
                        )
                    mean_ps = ps2.tile([128, NF], F32, tag="acc")
                    for k in range(8):
                        nc.tensor.matmul(
                            mean_ps[:, :N], ones128[:],
                            xbt[:, k * NF : k * NF + N],
                            start=(k == 0), stop=(k == 7),
                        )
                    sq_ps = ps2.tile([128, NF], F32, tag="acc")
                    for k in range(8):
                        sqt = lsb.tile([128, NF], BF16, tag="sq")
                        nc.scalar.activation(
                            sqt[:, :N], xbt[:, k * NF : k * NF + N], AF.Square
                        )
                        nc.tensor.matmul(
                            sq_ps[:, :N], ones128[:], sqt[:, :N],
                            start=(k == 0), stop=(k == 7),
                        )
                    m1 = lsb.tile([128, NF], F32, tag="m1")
                    nc.vector.tensor_scalar_mul(m1[:, :N], mean_ps[:, :N], 1.0 / D)
                    msq = lsb.tile([128, NF], F32, tag="msq")
                    nc.any.tensor_tensor(msq[:, :N], m1[:, :N], m1[:, :N], ALU.mult)
                    v = lsb.tile([128, NF], F32, tag="v")
                    nc.vector.scalar_tensor_tensor(
                        v[:, :N], sq_ps[:, :N], 1.0 / D, msq[:, :N],
                        ALU.mult, ALU.subtract,
                    )
                    s = lsb.tile([128, NF], F32, tag="s")
                    nc.scalar.activation(s[:, :N], v[:, :N], AF.Sqrt, bias=epsT[:])
                    r = lsb.tile([128, NF], F32, tag="r")
                    nc.vector.reciprocal(r[:, :N], s[:, :N])
                    for k in range(8):
                        ks = slice(k * NF, k * NF + N)
                        xs = lsb.tile([128, NF], F32, tag="xs")
                        nc.any.tensor_tensor(xs[:, :N], xp[:, ks], m1[:, :N], ALU.subtract)
                        nc.any.tensor_tensor(x[:, ks], xs[:, :N], r[:, :N], ALU.mult)
                        if flags["apply_gemb"]:
                            nc.vector.tensor_scalar(
                                x[:, ks], x[:, ks], gemb_sb[:, k : k + 1],
                                bemb_sb[:, k : k + 1], ALU.mult, ALU.add,
                            )

                    # ---- transformer layers ----
                    for l in range(L):
                        wq_sb = lw.tile([128, H * 128], BF16, tag="wq")
                        nc.sync.dma_start(wq_sb[:], wq_d[l])
                        wk_sb = lw.tile([128, H * 128], BF16, tag="wk")
                        nc.sync.dma_start(wk_sb[:], wk_d[l])
                        wv_sb = lw.tile([128, H * 128], BF16, tag="wv")
                        nc.sync.dma_start(wv_sb[:], wv_d[l])
                        wf1_sb = lw.tile([128, H * 8 * 128], BF16, tag="wf1")
                        nc.sync.dma_start(wf1_sb[:], wf1_d[l])
                        wf2_sb = lw.tile([128, H * 128], BF16, tag="wf2")
                        nc.sync.dma_start(wf2_sb[:], wf2_d[l])

                        xbt = xres.tile([128, 8 * NF], BF16, tag="xbt")
                        for k in range(8):
                            nc.any.tensor_copy(
                                xbt[:, k * NF : k * NF + N], x[:, k * NF : k * NF + N]
                            )
                        for h in range(8):
                            hs = slice(h * 128, (h + 1) * 128)
                            xh = xbt[:, h * NF : h * NF + N]
                            q_ps = ps3.tile([128, NF], F32, tag="qky")
                            nc.tensor.matmul(
                                q_ps[:, :N], wq_sb[:, hs], xh, start=True, stop=True
                            )
                            k_ps = ps3.tile([128, NF], F32, tag="qky")
                            nc.tensor.matmul(
                                k_ps[:, :N], wk_sb[:, hs], xh, start=True, stop=True
                            )
                            q_sb = lsb.tile([128, NF], BF16, tag="q")
                            nc.any.tensor_copy(q_sb[:, :N], q_ps[:, :N])
                            k_sb = lsb.tile([128, NF], BF16, tag="k")
                            nc.any.tensor_copy(k_sb[:, :N], k_ps[:, :N])
                            vt_ps = psv.tile([71, G * 128], F32, tag="vt")
                            for g in range(Gn):
                                nc.tensor.matmul(
                                    vt_ps[:, g * 128 : (g + 1) * 128],
                                    xbt[:, h * NF + g * T : h * NF + g * T + T],
                                    wv_sb[:, hs], start=True, stop=True,
                                )
                            vt_sb = lsb.tile([71, G * 128], BF16, tag="vt")
                            nc.any.tensor_copy(
                                vt_sb[:, : Gn * 128], vt_ps[:, : Gn * 128]
                            )
                            l_ps = ps2.tile([71, NF], F32, tag="acc")
                            for g in range(Gn):
                                gs = slice(g * T, (g + 1) * T)
                                nc.tensor.matmul(
                                    l_ps[:, gs], k_sb[:, gs], q_sb[:, gs],
                                    start=True, stop=True,
                                )
                            el = lsb.tile([71, NF], BF16, tag="el")
                            nc.scalar.activation(el[:, :N], l_ps[:, :N], AF.Exp)
                            cs_ps = ps2.tile([71, NF], F32, tag="acc")
                            nc.tensor.matmul(
                                cs_ps[:, :N], ones71[:, :71], el[:, :N],
                                start=True, stop=True,
                            )
                            r_sb = lsb.tile([71, NF], F32, tag="rr")
                            nc.vector.reciprocal(r_sb[:, :N], cs_ps[:, :N])
                            eln = lsb.tile([71, NF], BF16, tag="eln")
                            nc.vector.tensor_tensor(
                                eln[:, :N], el[:, :N], r_sb[:, :N], ALU.mult
                            )
                            y_ps = ps3.tile([128, NF], F32, tag="qky")
                            for g in range(Gn):
                                gs = slice(g * T, (g + 1) * T)
                                nc.tensor.matmul(
                                    y_ps[:, gs], vt_sb[:, g * 128 : (g + 1) * 128],
                                    eln[:, gs], start=True, stop=True,
                                )
                            nc.any.tensor_tensor(
                                xa[:, h * NF : h * NF + N], y_ps[:, :N],
                                x[:, h * NF : h * NF + N], ALU.add,
                            )
                            nc.any.tensor_copy(
                                xab[:, h * NF : h * NF + N],
                                xa[:, h * NF : h * NF + N],
                            )
                        y1 = big.tile([128, 8 * NF], BF16, tag="y1")
                        for m in range(8):
                            f_ps = ps2.tile([128, NF], F32, tag="acc")
                            for k in range(8):
                                nc.tensor.matmul(
                                    f_ps[:, :N],
                                    wf1_sb[:, (m * 8 + k) * 128 : (m * 8 + k + 1) * 128],
                                    xab[:, k * NF : k * NF + N],
                                    start=(k == 0), stop=(k == 7),
                                )
                            leaky(y1[:, m * NF : m * NF + N], f_ps[:, :N])
                        for h in range(8):
                            g_ps = ps2.tile([128, NF], F32, tag="acc")
                            nc.tensor.matmul(
                                g_ps[:, :N], wf2_sb[:, h * 128 : (h + 1) * 128],
                                y1[:, h * NF : h * NF + N], start=True, stop=True,
                            )
                            y2 = lsb.tile([128, NF], F32, tag="y2")
                            leaky(y2[:, :N], g_ps[:, :N])
                            nc.any.tensor_tensor(
                                x[:, h * NF : h * NF + N], y2[:, :N],
                                xa[:, h * NF : h * NF + N], ALU.add,
                            )

                    # ---- extract tokens 69/70 into u_d ----
                    for kt in range(16):
                        k = kt % 8
                        tok = 69 if kt < 8 else 70
                        srcap = (
                            x[:, k * NF : k * NF + N]
                            .rearrange("p (g t) -> p g t", t=T)[:, :, tok]
                        )
                        nc.sync.dma_start(u_d[kt][:, ucols], srcap)

                if NT > 0:
                    with tc.For_i(0, NT) as it:
                        tile_body(bass.ts(it, NF), bass.ts(it, G), NF, G)
                if REM > 0:
                    tile_body(
                        slice(NT * NF, NT * NF + NR),
                        slice(NT * G, NT * G + REM), NR, REM,
                    )

            # ---------------- head ----------------
            with (
                tc.tile_pool(name="h_sb", bufs=2) as hsb,
                tc.tile_pool(name="h_res", bufs=1) as hres,
                tc.tile_pool(name="h_ps", bufs=2, space="PSUM") as hps,
            ):
                u = hres.tile([128, 16 * Bc], F32)
                for k in range(16):
                    nc.sync.dma_start(u[:, k * Bc : (k + 1) * Bc], u_d[k])
                ub = hres.tile([128, 16 * Bc], BF16)
                for k in range(16):
                    ks = slice(k * Bc, (k + 1) * Bc)
                    nc.any.tensor_copy(ub[:, ks], u[:, ks])
                mean_ps = hps.tile([128, Bc], F32, tag="ln")
                for k in range(16):
                    nc.tensor.matmul(
                        mean_ps[:], ones128[:], ub[:, k * Bc : (k + 1) * Bc],
                        start=(k == 0), stop=(k == 15),
                    )
                sq_ps = hps.tile([128, Bc], F32, tag="ln")
                for k in range(16):
                    sqt = hsb.tile([128, Bc], BF16, tag="sq")
                    nc.scalar.activation(sqt[:], ub[:, k * Bc : (k + 1) * Bc], AF.Square)
                    nc.tensor.matmul(
                        sq_ps[:], ones128[:], sqt[:], start=(k == 0), stop=(k == 15)
                    )
                m1 = hsb.tile([128, Bc], F32, tag="m1")
                nc.vector.tensor_scalar_mul(m1[:], mean_ps[:], 1.0 / (2 * D))
                msq = hsb.tile([128, Bc], F32, tag="msq")
                nc.any.tensor_tensor(msq[:], m1[:], m1[:], ALU.mult)
                v = hsb.tile([128, Bc], F32, tag="v")
                nc.vector.scalar_tensor_tensor(
                    v[:], sq_ps[:], 1.0 / (2 * D), msq[:], ALU.mult, ALU.subtract
                )
                s = hsb.tile([128, Bc], F32, tag="s")
                nc.scalar.activation(s[:], v[:], AF.Sqrt, bias=epsT[:])
                r = hsb.tile([128, Bc], F32, tag="r")
                nc.vector.reciprocal(r[:], s[:])
                if flags["apply_ghead"]:
                    hg_sb = hres.tile([128, 16], F32)
                    nc.sync.dma_start(hg_sb[:], hg_d[:])
                    hbt_sb = hres.tile([128, 16], F32)
                    nc.sync.dma_start(hbt_sb[:], hbt_d[:])
                unb = hres.tile([128, 16 * Bc], BF16)
                for k in range(16):
                    ks = slice(k * Bc, (k + 1) * Bc)
                    xs = hsb.tile([128, Bc], F32, tag="xs")
                    nc.any.tensor_tensor(xs[:], u[:, ks], m1[:], ALU.subtract)
                    xn = hsb.tile([128, Bc], F32, tag="xn")
                    nc.any.tensor_tensor(xn[:], xs[:], r[:], ALU.mult)
                    if flags["apply_ghead"]:
                        nc.vector.tensor_scalar(
                            xn[:], xn[:], hg_sb[:, k : k + 1], hbt_sb[:, k : k + 1],
                            ALU.mult, ALU.add,
                        )
                    nc.any.tensor_copy(unb[:, ks], xn[:])
                w1_tiles = []
                for k in range(16):
                    wt = hres.tile([128, 2 * D], BF16, tag=f"w1_{k}")
                    nc.sync.dma_start(wt[:], w1t_d[k])
                    w1_tiles.append(wt)
                if flags["use_b1"]:
                    hb1_sb = hres.tile([128, 16], F32)
                    nc.sync.dma_start(hb1_sb[:], hb1_d[:])
                    hb1s_sb = hres.tile([128, 16], F32)
                    nc.sync.dma_start(hb1s_sb[:], hb1s_d[:])
                h1 = hres.tile([128, 16 * Bc], BF16)
                for m in range(16):
                    f_ps = hps.tile([128, Bc], F32, tag="f")
                    for k in range(16):
                        nc.tensor.matmul(
                            f_ps[:], w1_tiles[k][:, m * 128 : (m + 1) * 128],
                            unb[:, k * Bc : (k + 1) * Bc],
                            start=(k == 0), stop=(k == 15),
                        )
                    t1 = hsb.tile([128, Bc], F32, tag="t1")
                    ms = slice(m * Bc, (m + 1) * Bc)
                    if flags["use_b1"]:
                        nc.scalar.activation(
                            t1[:], f_ps[:], AF.Copy, scale=0.2, bias=hb1s_sb[:, m : m + 1]
                        )
                        s1 = hsb.tile([128, Bc], F32, tag="s1")
                        nc.vector.tensor_scalar_add(s1[:], f_ps[:], hb1_sb[:, m : m + 1])
                        nc.any.tensor_tensor(h1[:, ms], t1[:], s1[:], ALU.max)
                    else:
                        nc.scalar.activation(t1[:], f_ps[:], AF.Copy, scale=0.2)
                        nc.any.tensor_tensor(h1[:, ms], t1[:], f_ps[:], ALU.max)
                w2_sb = hres.tile([128, 16], BF16)
                nc.sync.dma_start(w2_sb[:], w2s_d[:])
                o_ps = hps.tile([1, Bc], F32, tag="o")
                for k in range(16):
                    nc.tensor.matmul(
                        o_ps[:], w2_sb[:, k : k + 1], h1[:, k * Bc : (k + 1) * Bc],
                        start=(k == 0), stop=(k == 15),
                    )
                o_sb = hsb.tile([1, Bc], F32, tag="o")
                if flags["use_b2"]:
                    b2_sb = hres.tile([1, 1], F32)
                    nc.sync.dma_start(b2_sb[:], b2_d[:])
                    nc.scalar.activation(o_sb[:], o_ps[:], AF.Sigmoid, bias=b2_sb[:])
                else:
                    nc.scalar.activation(o_sb[:], o_ps[:], AF.Sigmoid)
                nc.sync.dma_start(out_d[:], o_sb[:])

    return nc


TRACE = False
LAST_RESULT = None


def kernel(**inputs):
    global LAST_RESULT
    in_maps, flags = host_prep(inputs, N_CORES)
    nc = build_program(flags)
    nc.compile()
    res = run_bass_kernel_spmd(
        nc, in_maps, core_ids=list(range(N_CORES)), trace=TRACE
    )
    LAST_RESULT = res
    Bc = flags["Bc"]
    out = np.concatenate([res.results[c]["out"].reshape(Bc, 1) for c in range(N_CORES)])
    return out.astype(np.float32)
